# revision 5
# baseline (speedup 1.0000x reference)
"""GemmaAttention Trainium2 kernel (8 NeuronCores, sequence-parallel).

Strategy: shard queries over the 8 cores (each core owns S/8=256 query
positions of both batches, all 8 heads). Each core recomputes K/V for the
full sequence (cheap: Wk/Wv are D x 256), computes its block of
attn = softmax(256 * qk) (written out: [B,H,256,S] per core) and its rows of
out = attn_out @ Wo. No cross-core communication needed: both outputs are
disjoint per-core slices.

Precision: Q/K projections + QK^T run in true fp32 (the reference multiplies
scores by d_k=256, so softmax exponents are O(1e4) and need full fp32).
V projection, attn@V and the Wo projection run in fp32r (tf32-like, ~1e-4
rel) which is far inside the reference's own fp32 noise floor (~1e-2).
RoPE cos/sin tables are computed host-side with the same jnp formula as the
reference for bit-identical rounding.
"""

import numpy as np

try:
    import concourse.bass as bass  # noqa: F401
except Exception:
    import sys
    for _p in ("/opt/trn_rl_repo", "/root/.axon_site/_ro/trn_rl_repo"):
        if _p not in sys.path:
            sys.path.append(_p)

import concourse.bacc as bacc
import concourse.mybir as mybir
import concourse.tile as tile
from concourse import bass_utils
from concourse.masks import make_identity

P = 128          # partitions
B = 2            # batch
S = 2048         # sequence
D = 2048         # model dim
H = 8            # query heads
HD = 256         # head dim
NC = 8           # cores
CH = S // NC     # query rows per core per batch (256)
KC = D // P      # contraction chunks (16)
NW = S // P      # K/V windows per batch (16)
NS = S // 512    # score column chunks (4)

f32 = mybir.dt.float32
f32r = mybir.dt.float32r


def _build_nc(with_mask: bool):
    nc = bacc.Bacc()

    hT = nc.dram_tensor("hT", [D, B, S], f32, kind="ExternalInput")
    hTq = nc.dram_tensor("hTq", [D, B, CH], f32, kind="ExternalInput")
    Wq = nc.dram_tensor("Wq", [D, D], f32, kind="ExternalInput")
    Wk = nc.dram_tensor("Wk", [D, HD], f32, kind="ExternalInput")
    Wv = nc.dram_tensor("Wv", [D, HD], f32, kind="ExternalInput")
    Wo = nc.dram_tensor("Wo", [D, D], f32, kind="ExternalInput")
    cosk = nc.dram_tensor("cosk", [P, B, S], f32, kind="ExternalInput")
    sink = nc.dram_tensor("sink", [P, B, S], f32, kind="ExternalInput")
    cosq = nc.dram_tensor("cosq", [P, B, CH], f32, kind="ExternalInput")
    sinq = nc.dram_tensor("sinq", [P, B, CH], f32, kind="ExternalInput")
    if with_mask:
        mask_c = nc.dram_tensor("mask_c", [B, CH, S], f32, kind="ExternalInput")

    attn_c = nc.dram_tensor("attn_c", [B, H, CH, S], f32, kind="ExternalOutput")
    out_c = nc.dram_tensor("out_c", [B, CH, D], f32, kind="ExternalOutput")

    with tile.TileContext(nc) as tc:
        with tc.tile_pool(name="persist", bufs=1) as persist, \
             tc.tile_pool(name="tmp", bufs=4) as tmp, \
             tc.tile_pool(name="pp", bufs=4, space="PSUM") as pp, \
             tc.tile_pool(name="scp", bufs=1, space="PSUM") as scp:

            ident = persist.tile([P, P], f32)
            make_identity(nc, ident)
            cq = persist.tile([P, B, CH], f32)
            sq = persist.tile([P, B, CH], f32)
            nc.sync.dma_start(cq, cosq[:, :, :])
            nc.sync.dma_start(sq, sinq[:, :, :])
            # roped q, laid out [hd-chunk partitions, chunk idx, b, q]
            qT = persist.tile([P, KC, B, CH], f32)
            # attn-out transposed [hd-chunk partitions, chunk idx, (b q)]
            aoT = persist.tile([P, KC, B * CH], f32r)

            # ---------------- Q projection + RoPE -------------------------
            with tc.tile_pool(name="qp", bufs=1) as qp:
                hTq_sb = qp.tile([P, KC, B, CH], f32, tag="hTq", bufs=1)
                nc.sync.dma_start(
                    hTq_sb, hTq[:, :, :].rearrange("(kc p) b q -> p kc b q", p=P))
                for h in range(H):
                    psums = []
                    for j in range(2):
                        m = 2 * h + j
                        wqb = qp.tile([P, KC, P], f32, tag="wqb", bufs=2)
                        nc.sync.dma_start(
                            wqb,
                            Wq[:, m * P:(m + 1) * P].rearrange(
                                "(kc p) m -> p kc m", p=P))
                        pq = pp.tile([P, 512], f32, tag="pp", name="pp")
                        for kc in range(KC):
                            nc.tensor.matmul(pq, wqb[:, kc], hTq_sb[:, kc],
                                             start=(kc == 0), stop=(kc == KC - 1))
                        psums.append(pq)
                    p0 = psums[0].rearrange("p (b q) -> p b q", b=B)
                    p1 = psums[1].rearrange("p (b q) -> p b q", b=B)
                    t0 = tmp.tile([P, B, CH], f32, tag="t0", bufs=2)
                    t1 = tmp.tile([P, B, CH], f32, tag="t1", bufs=2)
                    nc.vector.tensor_mul(t0, p0, cq)
                    nc.vector.tensor_mul(t1, p1, sq)
                    nc.vector.tensor_sub(qT[:, 2 * h], t0, t1)
                    t2 = tmp.tile([P, B, CH], f32, tag="t2", bufs=2)
                    t3 = tmp.tile([P, B, CH], f32, tag="t3", bufs=2)
                    nc.vector.tensor_mul(t2, p1, cq)
                    nc.vector.tensor_mul(t3, p0, sq)
                    nc.vector.tensor_add(qT[:, 2 * h + 1], t2, t3)

            for b in range(B):
                # roped K^T for batch b: [hd-chunk partitions, chunk, s]
                ktb = persist.tile([P, 2, S], f32, tag="ktb", bufs=1)
                # V for batch b: [s-row partitions, window, hd]
                vb = persist.tile([P, NW, HD], f32r, tag="vb", bufs=1)

                # ------------- K/V projection for batch b -----------------
                with tc.tile_pool(name="kvw", bufs=1) as kvw, \
                     tc.tile_pool(name="winp", bufs=2) as winp:
                    wk = kvw.tile([P, KC, HD], f32, tag="wk", bufs=1)
                    wv = kvw.tile([P, KC, HD], f32r, tag="wv", bufs=1)
                    nc.sync.dma_start(
                        wk, Wk[:, :].rearrange("(kc p) n -> p kc n", p=P))
                    nc.sync.dma_start(
                        wv, Wv[:, :].rearrange("(kc p) n -> p kc n", p=P).bitcast(f32r))
                    for w in range(NW):
                        sl = slice(w * P, (w + 1) * P)
                        win = winp.tile([P, KC, P], f32, tag="win", bufs=2)
                        win_r = winp.tile([P, KC, P], f32r, tag="win_r", bufs=2)
                        src = hT[:, b, sl].rearrange("(kc p) s -> p kc s", p=P)
                        nc.sync.dma_start(win, src)
                        nc.sync.dma_start(win_r, src.bitcast(f32r))
                        cwin = winp.tile([P, P], f32, tag="cwin", bufs=2)
                        swin = winp.tile([P, P], f32, tag="swin", bufs=2)
                        nc.sync.dma_start(cwin, cosk[:, b, sl])
                        nc.sync.dma_start(swin, sink[:, b, sl])

                        pk0 = pp.tile([P, 512], f32, tag="pp", name="pp")[:, :P]
                        pk1 = pp.tile([P, 512], f32, tag="pp", name="pp")[:, :P]
                        for kc in range(KC):
                            nc.tensor.matmul(pk0, wk[:, kc, 0:P], win[:, kc],
                                             start=(kc == 0), stop=(kc == KC - 1))
                        for kc in range(KC):
                            nc.tensor.matmul(pk1, wk[:, kc, P:HD], win[:, kc],
                                             start=(kc == 0), stop=(kc == KC - 1))
                        ta = tmp.tile([P, P], f32, tag="ta", bufs=2)
                        tb_ = tmp.tile([P, P], f32, tag="tb", bufs=2)
                        nc.vector.tensor_mul(ta, pk0, cwin)
                        nc.vector.tensor_mul(tb_, pk1, swin)
                        nc.vector.tensor_sub(ktb[:, 0, sl], ta, tb_)
                        tcc = tmp.tile([P, P], f32, tag="tc", bufs=2)
                        td = tmp.tile([P, P], f32, tag="td", bufs=2)
                        nc.vector.tensor_mul(tcc, pk1, cwin)
                        nc.vector.tensor_mul(td, pk0, swin)
                        nc.vector.tensor_add(ktb[:, 1, sl], tcc, td)

                        pv = pp.tile([P, 512], f32, tag="pp", name="pp")[:, :HD]
                        for kc in range(KC):
                            nc.tensor.matmul(pv, win_r[:, kc], wv[:, kc],
                                             start=(kc == 0), stop=(kc == KC - 1))
                        nc.vector.tensor_copy(vb[:, w], pv)

                # ------------- attention for batch b ----------------------
                with tc.tile_pool(name="attp", bufs=2) as attp, \
                     tc.tile_pool(name="attst", bufs=4) as attst:
                    for h in range(H):
                        probsT = attp.tile([P, NW, 2 * P], f32r,
                                           tag="probsT", bufs=2)
                        for t in range(2):
                            tsl = slice(t * P, (t + 1) * P)
                            sc = scp.tile([P, S], f32, tag="sc", bufs=1)
                            s_sb = attp.tile([P, S], f32, tag="s_sb", bufs=2)
                            mx = attst.tile([P, NS], f32, tag="mx", bufs=4)
                            if with_mask:
                                mk = attp.tile([P, S], f32, tag="mk", bufs=2)
                                nc.sync.dma_start(
                                    mk, mask_c[b, t * P:(t + 1) * P, :])
                            for n in range(NS):
                                nsl = slice(n * 512, (n + 1) * 512)
                                for c in range(2):
                                    nc.tensor.matmul(
                                        sc[:, nsl],
                                        qT[:, 2 * h + c, b, tsl],
                                        ktb[:, c, nsl],
                                        start=(c == 0), stop=(c == 1))
                                nc.any.tensor_scalar_mul(
                                    s_sb[:, nsl], sc[:, nsl], 256.0)
                                if with_mask:
                                    nc.vector.tensor_add(
                                        s_sb[:, nsl], s_sb[:, nsl], mk[:, nsl])
                                nc.vector.reduce_max(
                                    out=mx[:, n:n + 1], in_=s_sb[:, nsl],
                                    axis=mybir.AxisListType.X)
                            rowmax = attst.tile([P, 1], f32, tag="rmx", bufs=4)
                            nc.vector.reduce_max(out=rowmax, in_=mx,
                                                 axis=mybir.AxisListType.X)
                            nbias = attst.tile([P, 1], f32, tag="nb", bufs=4)
                            nc.vector.tensor_scalar_mul(nbias, rowmax, -1.0)
                            probs = attp.tile([P, S], f32, tag="probs", bufs=2)
                            denom = attst.tile([P, 1], f32, tag="dn", bufs=4)
                            nc.scalar.activation(
                                out=probs, in_=s_sb,
                                func=mybir.ActivationFunctionType.Exp,
                                bias=nbias, scale=1.0, accum_out=denom)
                            rden = attst.tile([P, 1], f32, tag="rd", bufs=4)
                            nc.vector.reciprocal(rden, denom)
                            nc.vector.tensor_scalar_mul(probs, probs, rden)
                            nc.sync.dma_start(attn_c[b, h, tsl, :], probs)
                            for kc in range(NW):
                                pt = pp.tile([P, 512], f32, tag="pp", name="pp")[:, :P]
                                nc.tensor.transpose(
                                    pt, probs[:, kc * P:(kc + 1) * P], ident)
                                nc.vector.tensor_copy(probsT[:, kc, tsl], pt)
                        for m in range(2):
                            pav = pp.tile([P, 512], f32, tag="pp", name="pp")[:, :HD]
                            for w in range(NW):
                                nc.tensor.matmul(
                                    pav, vb[:, w, m * P:(m + 1) * P],
                                    probsT[:, w],
                                    start=(w == 0), stop=(w == NW - 1))
                            nc.vector.tensor_copy(
                                aoT[:, 2 * h + m, b * CH:(b + 1) * CH], pav)

            # ---------------- output projection ---------------------------
            with tc.tile_pool(name="wop", bufs=1) as wop:
                for n in range(NS):
                    nsl = slice(n * 512, (n + 1) * 512)
                    wob = wop.tile([P, KC, 512], f32r, tag="wob", bufs=2)
                    nc.sync.dma_start(
                        wob,
                        Wo[:, nsl].rearrange("(kc p) d -> p kc d", p=P).bitcast(f32r))
                    for mq in range(4):
                        bq, tq = divmod(mq, 2)
                        pw = pp.tile([P, 512], f32, tag="pp", name="pp")
                        for kc in range(KC):
                            nc.tensor.matmul(
                                pw, aoT[:, kc, mq * P:(mq + 1) * P], wob[:, kc],
                                start=(kc == 0), stop=(kc == KC - 1))
                        osb = wop.tile([P, 512], f32, tag="osb", bufs=3)
                        nc.vector.tensor_copy(osb, pw)
                        nc.sync.dma_start(
                            out_c[bq, tq * P:(tq + 1) * P, nsl], osb)

    nc.compile()
    return nc


_NC_CACHE = {}


def _get_nc(with_mask: bool):
    if with_mask not in _NC_CACHE:
        _NC_CACHE[with_mask] = _build_nc(with_mask)
    return _NC_CACHE[with_mask]


def _rope_tables(position_ids):
    """cos/sin tables [B, S, HD/2] with the reference's exact f32 rounding."""
    import jax
    import jax.numpy as jnp
    with jax.default_device(jax.devices("cpu")[0]):
        inv_freq = 1.0 / (10000.0 ** (jnp.arange(0, HD, 2, dtype=jnp.float32) / HD))
        freqs = jnp.asarray(position_ids).astype(jnp.float32)[..., None] * inv_freq
        cos = np.asarray(jnp.cos(freqs))
        sin = np.asarray(jnp.sin(freqs))
    return cos, sin


def kernel(hidden_states, attention_mask, position_ids, Wq, Wk, Wv, Wo):
    hidden_states = np.asarray(hidden_states)
    attention_mask = np.asarray(attention_mask)
    position_ids = np.asarray(position_ids)
    Wq = np.ascontiguousarray(np.asarray(Wq, np.float32))
    Wk = np.ascontiguousarray(np.asarray(Wk, np.float32))
    Wv = np.ascontiguousarray(np.asarray(Wv, np.float32))
    Wo = np.ascontiguousarray(np.asarray(Wo, np.float32))

    with_mask = bool(np.any(attention_mask))
    nc = _get_nc(with_mask)

    # host-side layout prep (no model FLOPs: transpose + rope tables)
    hT = np.ascontiguousarray(hidden_states.astype(np.float32).transpose(2, 0, 1))
    cos, sin = _rope_tables(position_ids)          # [B, S, 128]
    coskT = np.ascontiguousarray(cos.transpose(2, 0, 1))  # [128, B, S]
    sinkT = np.ascontiguousarray(sin.transpose(2, 0, 1))

    in_maps = []
    for c in range(NC):
        qsl = slice(c * CH, (c + 1) * CH)
        m = {
            "hT": hT,
            "hTq": np.ascontiguousarray(hT[:, :, qsl]),
            "Wq": Wq, "Wk": Wk, "Wv": Wv, "Wo": Wo,
            "cosk": coskT, "sink": sinkT,
            "cosq": np.ascontiguousarray(coskT[:, :, qsl]),
            "sinq": np.ascontiguousarray(sinkT[:, :, qsl]),
        }
        if with_mask:
            m["mask_c"] = np.ascontiguousarray(
                attention_mask[:, 0, qsl, :].astype(np.float32))
        in_maps.append(m)

    res = bass_utils.run_bass_kernel_spmd(
        nc, in_maps, core_ids=list(range(NC)), trace=False)

    out = np.empty((B, S, D), np.float32)
    attn = np.empty((B, H, S, S), np.float32)
    for c in range(NC):
        qsl = slice(c * CH, (c + 1) * CH)
        out[:, qsl, :] = res.results[c]["out_c"]
        attn[:, :, qsl, :] = res.results[c]["attn_c"]
    return out, attn
